# revision 7
# baseline (speedup 1.0000x reference)
"""Trainium2 Bass kernel for a ChannelAttention module.

Reference computation (per row b of B = 2048 rows, each row is (n=64, c=512)):
    y  = mean_c x                      # (B, 64)
    lr = y @ w1.T + b1                 # (B, 32)
    f1 = lr @ mb                       # (B, 128)
    at = softmax(f1 / sqrt(32))        # (B, 128)
    y1 = at @ mb.T                     # (B, 32)
    y2 = sigmoid(y1 @ w2.T + b2)       # (B, 64)
    out = x * y2[..., None]

Memory-bound: 256 MiB in + 256 MiB out. Strategy: data-parallel over 8 cores
(256 rows each), single streaming pass per core. The two inner linears fold
host-side into two small fused matrices so the on-chip MLP is:
    f1_raw = y_sum @ A          A = (w1.T @ mb) / 512          [64, 128]
    e      = exp(f1_raw*s + be) be = (b1 @ mb) * s, s=32^-0.5  [128, 1]
    [z|S]  = Daug.T @ e         Daug = [(w2 @ mb).T | ones]    [128, 65]
    y2     = sigmoid(z / S + b2)
(softmax max-subtraction is skipped: |f1*s| < ~3 for these magnitudes, and the
result is mathematically identical.)

SBUF layout: x streamed as [128, 512] tiles = 2 rows per tile, partition
p = r*64 + j (r = row parity, j = channel). The c-reduction lands in
y_coll[128, G]; its partition halves ARE the transposed-MLP operand
yT [j, col] for even/odd rows, so no on-chip transpose is ever needed.

bf16 I/O: the rel-err budget (2e-2) dwarfs bf16 rounding (~2e-3), so the
host converts x to bf16 before staging and the kernel streams/scales/stores
bf16 — halving device HBM traffic (64 -> 32 MiB per core). The tiny MLP
stays fp32 (reduce_sum emits fp32 from bf16 input).
"""

import os
import sys

import numpy as np

for _p in ("/opt/trn_rl_repo",):
    if _p not in sys.path:
        sys.path.insert(0, _p)

from contextlib import ExitStack

import ml_dtypes

from concourse import bacc, mybir, tile
from concourse.bass_utils import run_bass_kernel_spmd

N_CORES = 8
ROWS = 2048              # total B rows
C = 512
N = 64
P = 128
TILES = (ROWS // N_CORES) // 2   # 128 [128, 512] tiles per core, 2 rows each
G = 16                           # tiles per MLP chunk
FP = mybir.dt.float32
XDT = mybir.dt.bfloat16          # streamed-x dtype (DRAM + SBUF x tiles)
XNP = ml_dtypes.bfloat16
SCALE = float(32 ** -0.5)
TPD = 8          # tiles (128 KiB each in bf16) per DMA transfer
HOST_PERM = True  # host pre-permutes shards so every DMA is contiguous

_CACHED = None
LAST_RESULTS = None  # BassKernelResults of the most recent kernel() call


def _build_module(
    tiles=TILES,
    g=G,
    repeat=1,
    tpd=TPD,
    store_engine="sync",
    xbufs=12,
    direct_scale=False,
    sv_engine="vector",
    sv_batch=True,
    mul_engine="scalar",
    mlp_bufs=2,
    host_perm=HOST_PERM,
    fine_tail=False,
):
    """repeat>1 wraps the streaming pass in an on-device For_i loop —
    used only for differential exec-time measurement (dispatch overhead
    cancels between two repeat counts).

    tpd = tiles per DMA: each load/store moves tpd*256KiB in one dma_start
    (3D access pattern [p, tpd, c]); bigger transfers amortize the per-DMA
    fixed cost. Loads issue on the SP HWDGE ring (nc.sync), stores on the
    ACT ring (nc.scalar) so the two streams don't share one FIFO."""
    nchunk = tiles // g
    assert g % tpd == 0
    nc = bacc.Bacc("TRN2", target_bir_lowering=False, debug=False)

    # host_perm: the host pre-permutes each shard to [tiles//tpd, P, tpd*C]
    # (group-major, partition-major) so every load/store is a fully
    # contiguous 2D AP — tpd*2KiB per partition per descriptor instead of
    # tpd separate 2KiB runs. The SBUF-side layout is identical.
    if host_perm:
        x_d = nc.dram_tensor("x", [tiles // tpd, P, tpd * C], XDT, kind="ExternalInput")
    else:
        x_d = nc.dram_tensor("x", [tiles, P, C], XDT, kind="ExternalInput")
    a_d = nc.dram_tensor("amat", [N, P], FP, kind="ExternalInput")
    be_d = nc.dram_tensor("bexp", [P, 1], FP, kind="ExternalInput")
    dg_d = nc.dram_tensor("daug", [P, N + 1], FP, kind="ExternalInput")
    b2_d = nc.dram_tensor("b2", [N, 1], FP, kind="ExternalInput")
    if host_perm:
        o_d = nc.dram_tensor("out", [tiles // tpd, P, tpd * C], XDT, kind="ExternalOutput")
    else:
        o_d = nc.dram_tensor("out", [tiles, P, C], XDT, kind="ExternalOutput")

    with tile.TileContext(nc) as tc, ExitStack() as ctx:
        const = ctx.enter_context(tc.tile_pool(name="const", bufs=1))
        xp = ctx.enter_context(
            tc.tile_pool(name="xp", bufs=xbufs or (2 * g // tpd))
        )
        yp = ctx.enter_context(tc.tile_pool(name="yp", bufs=mlp_bufs))
        sp = ctx.enter_context(tc.tile_pool(name="sp", bufs=mlp_bufs))
        svp = ctx.enter_context(tc.tile_pool(name="svp", bufs=2 * g))
        # 3 PSUM tags (f1/zs/rb) x bufs must fit 8 banks -> cap at 2
        pp = ctx.enter_context(
            tc.tile_pool(name="pp", bufs=min(mlp_bufs, 2), space="PSUM")
        )

        a_sb = const.tile([N, P], FP)
        nc.sync.dma_start(a_sb[:], a_d[:])
        be_sb = const.tile([P, 1], FP)
        nc.sync.dma_start(be_sb[:], be_d[:])
        dg_sb = const.tile([P, N + 1], FP)
        nc.sync.dma_start(dg_sb[:], dg_d[:])
        b2_sb = const.tile([N, 1], FP)
        nc.sync.dma_start(b2_sb[:], b2_d[:])
        ones_sb = const.tile([1, N], FP)
        nc.vector.memset(ones_sb[:], 1.0)

        loop_cm = tc.For_i(0, repeat, 1) if repeat > 1 else None
        if loop_cm is not None:
            loop_cm.__enter__()

        st_eng = {"scalar": nc.scalar, "sync": nc.sync, "gpsimd": nc.gpsimd}[
            store_engine
        ]
        for ch in range(nchunk):
            y_coll = yp.tile([P, g], FP)
            xts = []
            for i in range(0, g, tpd):
                t = ch * g + i
                xt = xp.tile([P, tpd * C], XDT)
                xt3 = xt[:].rearrange("p (d c) -> p d c", d=tpd)
                if host_perm:
                    nc.sync.dma_start(xt[:], x_d[t // tpd])
                else:
                    nc.sync.dma_start(
                        xt3, x_d[t : t + tpd].rearrange("d p c -> p d c")
                    )
                nc.vector.reduce_sum(
                    y_coll[:, i : i + tpd], xt3, axis=mybir.AxisListType.X
                )
                xts.append(xt)

            # y_coll halves are yT for even/odd rows: pack to [64, 2g]
            y_all = sp.tile([N, 2 * g], FP)
            nc.vector.tensor_copy(y_all[:, 0:g], y_coll[0:N, :])
            nc.vector.tensor_copy(y_all[:, g : 2 * g], y_coll[N:P, :])

            f1 = pp.tile([P, 2 * g], FP)
            nc.tensor.matmul(f1[:], a_sb[:], y_all[:])
            e_sb = sp.tile([P, 2 * g], FP)
            nc.scalar.activation(
                e_sb[:], f1[:], mybir.ActivationFunctionType.Exp,
                bias=be_sb[:], scale=SCALE,
            )
            zs = pp.tile([N + 1, 2 * g], FP)
            nc.tensor.matmul(zs[:], dg_sb[:], e_sb[:])
            rs = sp.tile([1, 2 * g], FP)
            nc.vector.reciprocal(rs[:], zs[N : N + 1, :])
            rb = pp.tile([N, 2 * g], FP)
            nc.tensor.matmul(rb[:], ones_sb[:], rs[:])
            rb_sb = sp.tile([N, 2 * g], FP)
            nc.scalar.copy(rb_sb[:], rb[:])
            zn = sp.tile([N, 2 * g], FP)
            nc.vector.tensor_mul(zn[:], zs[0:N, :], rb_sb[:])
            y2 = sp.tile([N, 2 * g], FP)
            nc.scalar.activation(
                y2[:], zn[:], mybir.ActivationFunctionType.Sigmoid, bias=b2_sb[:]
            )

            svc = None
            if sv_batch and not direct_scale:
                # all g per-tile scale vectors assembled in two copies:
                # svc[(r,j), i] = y2[j, r*g + i]
                sv_eng = getattr(nc, sv_engine)
                svc = svp.tile([P, g], FP)
                sv_eng.tensor_copy(svc[0:N, :], y2[:, 0:g])
                sv_eng.tensor_copy(svc[N:P, :], y2[:, g : 2 * g])

            for i in range(0, g, tpd):
                t = ch * g + i
                xt = xts[i // tpd]
                for u in range(tpd):
                    col = xt[:, u * C : (u + 1) * C]
                    if mul_engine == "scalar" or (
                        mul_engine == "mixed" and (i // tpd) % 2 == 0
                    ):
                        mul_eng = nc.scalar
                    elif mul_engine == "vector":
                        mul_eng = nc.vector
                    else:
                        mul_eng = nc.gpsimd
                    if svc is not None:
                        if mul_eng is nc.scalar:
                            nc.scalar.activation(
                                col, col,
                                mybir.ActivationFunctionType.Copy,
                                scale=svc[:, i + u : i + u + 1],
                            )
                        else:
                            mul_eng.tensor_scalar_mul(
                                col, col, svc[:, i + u : i + u + 1]
                            )
                    elif direct_scale:
                        # two half-partition muls reading y2 columns as the
                        # per-partition scale directly (no sv assembly)
                        nc.scalar.activation(
                            col[0:N, :], col[0:N, :],
                            mybir.ActivationFunctionType.Copy,
                            scale=y2[:, i + u : i + u + 1],
                        )
                        nc.scalar.activation(
                            col[N:P, :], col[N:P, :],
                            mybir.ActivationFunctionType.Copy,
                            scale=y2[:, g + i + u : g + i + u + 1],
                        )
                    else:
                        sv_eng = getattr(nc, sv_engine)
                        sv = svp.tile([P, 1], FP)
                        sv_eng.tensor_copy(sv[0:N, :], y2[:, i + u : i + u + 1])
                        sv_eng.tensor_copy(
                            sv[N:P, :], y2[:, g + i + u : g + i + u + 1]
                        )
                        nc.scalar.activation(
                            col, col,
                            mybir.ActivationFunctionType.Copy,
                            scale=sv[:],
                        )
                if host_perm:
                    if fine_tail and ch == nchunk - 1:
                        # last chunk: stream stores out in 2-tile pieces as
                        # their muls land, shortening the serial kernel tail
                        for s0 in range(0, tpd, 2):
                            st_eng.dma_start(
                                o_d[t // tpd][:, s0 * C : (s0 + 2) * C],
                                xt[:, s0 * C : (s0 + 2) * C],
                            )
                    else:
                        st_eng.dma_start(o_d[t // tpd], xt[:])
                else:
                    st_eng.dma_start(
                        o_d[t : t + tpd].rearrange("d p c -> p d c"),
                        xt[:].rearrange("p (d c) -> p d c", d=tpd),
                    )

        if loop_cm is not None:
            loop_cm.__exit__(None, None, None)

    nc.compile()
    return nc


def _prep_weights(w1, b1, w2, b2, mb):
    w1 = np.asarray(w1, np.float64)
    b1 = np.asarray(b1, np.float64)
    w2 = np.asarray(w2, np.float64)
    b2 = np.asarray(b2, np.float64)
    mb = np.asarray(mb, np.float64)
    a = np.ascontiguousarray(((w1.T @ mb) / C).astype(np.float32))
    be = np.ascontiguousarray(((b1 @ mb) * SCALE).astype(np.float32).reshape(P, 1))
    dg = np.concatenate([(w2 @ mb).T, np.ones((P, 1))], axis=1)
    dg = np.ascontiguousarray(dg.astype(np.float32))
    b2c = np.ascontiguousarray(b2.astype(np.float32).reshape(N, 1))
    return a, be, dg, b2c


def prep_x_shards(x):
    """fp32 (b,N,Nwin,p,n,c) -> per-core bf16 shards in the module's layout."""
    xs = np.asarray(x, np.float32).reshape(N_CORES, TILES, P, C).astype(XNP)
    if HOST_PERM:
        # group-major, partition-major packing: every on-device DMA becomes
        # one contiguous TPD*1KiB run per partition (see _build_module)
        xs = np.ascontiguousarray(
            xs.reshape(N_CORES, TILES // TPD, TPD, P, C).transpose(0, 1, 3, 2, 4)
        ).reshape(N_CORES, TILES // TPD, P, TPD * C)
    return xs


def prep_in_maps(x, w1, b1, w2, b2, mb):
    a, be, dg, b2c = _prep_weights(w1, b1, w2, b2, mb)
    xs = prep_x_shards(x)
    return [
        {"x": xs[i], "amat": a, "bexp": be, "daug": dg, "b2": b2c}
        for i in range(N_CORES)
    ]


def kernel(x, w1, b1, w2, b2, mb):
    global _CACHED
    x = np.asarray(x, np.float32)
    b, Nn, Nwin, p, n, c = x.shape

    if _CACHED is None:
        _CACHED = _build_module()
    nc = _CACHED

    in_maps = prep_in_maps(x, w1, b1, w2, b2, mb)
    global LAST_RESULTS
    LAST_RESULTS = run_bass_kernel_spmd(
        nc, in_maps, core_ids=list(range(N_CORES)),
        trace=bool(os.environ.get("KERNEL_TRACE")),
    )
    res = LAST_RESULTS.results
    out = np.stack([r["out"] for r in res], axis=0)
    if HOST_PERM:
        out = np.ascontiguousarray(
            out.reshape(N_CORES, TILES // TPD, P, TPD, C).transpose(0, 1, 3, 2, 4)
        )
    return out.reshape(b, Nn, Nwin, p, n, c).astype(np.float32)


if __name__ == "__main__":
    xt = np.random.randn(2, 16, 16, 4, 64, 512).astype(np.float32)
    w1t = (np.random.randn(32, 64) * 0.1).astype(np.float32)
    b1t = (np.random.randn(32) * 0.1).astype(np.float32)
    w2t = (np.random.randn(64, 32) * 0.1).astype(np.float32)
    b2t = (np.random.randn(64) * 0.1).astype(np.float32)
    mbt = np.random.randn(32, 128).astype(np.float32)
    o = kernel(xt, w1t, b1t, w2t, b2t, mbt)
    print(o.shape, o.dtype)



# revision 33
# speedup vs baseline: 1.3716x; 1.3716x over previous
"""Trainium2 Bass kernel for a ChannelAttention module.

Reference computation (per row b of B = 2048 rows, each row is (n=64, c=512)):
    y  = mean_c x                      # (B, 64)
    lr = y @ w1.T + b1                 # (B, 32)
    f1 = lr @ mb                       # (B, 128)
    at = softmax(f1 / sqrt(32))        # (B, 128)
    y1 = at @ mb.T                     # (B, 32)
    y2 = sigmoid(y1 @ w2.T + b2)       # (B, 64)
    out = x * y2[..., None]

Memory-bound: 256 MiB in + 256 MiB out. Strategy: data-parallel over 8 cores
(256 rows each), single streaming pass per core. The two inner linears fold
host-side into two small fused matrices so the on-chip MLP is:
    f1_raw = y_sum @ A          A = (w1.T @ mb) / 512          [64, 128]
    e      = exp(f1_raw*s + be) be = (b1 @ mb) * s, s=32^-0.5  [128, 1]
    [z|S]  = Daug.T @ e         Daug = [(w2 @ mb).T | ones]    [128, 65]
    y2     = sigmoid(z / S + b2)
(softmax max-subtraction is skipped: |f1*s| < ~3 for these magnitudes, and the
result is mathematically identical.)

SBUF layout: x streamed as [128, 512] tiles = 2 rows per tile, partition
p = r*64 + j (r = row parity, j = channel). The c-reduction lands in
y_coll[128, G]; its partition halves ARE the transposed-MLP operand
yT [j, col] for even/odd rows, so no on-chip transpose is ever needed.

bf16 I/O: the rel-err budget (2e-2) dwarfs bf16 rounding (~2e-3), so the
host converts x to bf16 before staging and the kernel streams/scales/stores
bf16 — halving device HBM traffic (64 -> 32 MiB per core). The tiny MLP
stays fp32 (reduce_sum emits fp32 from bf16 input).

Engine balance (8-core-concurrent DMA floor is ~108us for 32 MiB/core):
the ACT engine must not host the whole scale-multiply AND the MLP's
transcendentals. Sigmoid is recast as 1/(1+exp(-v)) so ACT only ever uses
the Exp table (Copy shares it; Sigmoid would force two 1.28us table loads
per chunk), the PSUM evacuation + reciprocal glue live on DVE, and the 16
per-chunk scale-muls are split 'split:K': K tiles as ACT per-column Copy
(per-partition scale AP), the rest as one DVE tensor_mul with the scale
broadcast along c via a stride-0 AP.
"""

import os
import sys

import numpy as np

for _p in ("/opt/trn_rl_repo",):
    if _p not in sys.path:
        sys.path.insert(0, _p)

from contextlib import ExitStack

import ml_dtypes

from concourse import bacc, mybir, tile
from concourse.bass_utils import run_bass_kernel_spmd

N_CORES = 8
ROWS = 2048              # total B rows
C = 512
N = 64
P = 128
TILES = (ROWS // N_CORES) // 2   # 128 [128, 512] tiles per core, 2 rows each
G = 16                           # tiles per MLP chunk
FP = mybir.dt.float32
XDT = mybir.dt.bfloat16          # streamed-x dtype (DRAM + SBUF x tiles)
XNP = ml_dtypes.bfloat16
SCALE = float(32 ** -0.5)
TPD = 8          # tiles (128 KiB each in bf16) per DMA transfer
HOST_PERM = True  # host pre-permutes shards so every DMA is contiguous
MUL_ENGINE = "split:12"  # 'scalar' | 'vector' | 'slab' | 'split:K' | ...
LOWP_SV = False     # bf16 svd scale tile for the DVE slab muls
LOWP_YCOLL = False  # bf16 y_coll reduce output
SIGMOID_EXP = True  # sigmoid via 1/(1+exp(-v)): keeps ACT on the Exp table
                    # (Exp+Copy share a table; Sigmoid would force 2 table
                    # loads per chunk at 1.28us each)

_CACHED = None
LAST_RESULTS = None  # BassKernelResults of the most recent kernel() call


def _build_module(
    tiles=TILES,
    g=G,
    repeat=1,
    tpd=TPD,
    store_engine="sync",
    xbufs=12,
    direct_scale=False,
    sv_engine="vector",
    sv_batch=True,
    mul_engine=None,
    mlp_bufs=2,
    host_perm=HOST_PERM,
    fine_tail=False,
    ablate=None,        # None | 'copy' | 'nomul' | 'constmul'  (diagnostics)
    dma_bitcast=None,   # DMA x/out as fp32-typed APs (same bytes)
    lowp_sv=None,       # bf16 svd scale tile (2x DVE slab throughput)
    lowp_ycoll=None,    # bf16 y_coll reduce output (2x DVE reduce throughput)
):
    if lowp_sv is None:
        lowp_sv = LOWP_SV
    if lowp_ycoll is None:
        lowp_ycoll = LOWP_YCOLL
    if mul_engine is None:
        mul_engine = MUL_ENGINE
    if dma_bitcast is None:
        dma_bitcast = DMA_BITCAST
    """repeat>1 wraps the streaming pass in an on-device For_i loop —
    used only for differential exec-time measurement (dispatch overhead
    cancels between two repeat counts).

    tpd = tiles per DMA: each load/store moves tpd*256KiB in one dma_start
    (3D access pattern [p, tpd, c]); bigger transfers amortize the per-DMA
    fixed cost. Loads issue on the SP HWDGE ring (nc.sync), stores on the
    ACT ring (nc.scalar) so the two streams don't share one FIFO."""
    nchunk = tiles // g
    assert g % tpd == 0
    nc = bacc.Bacc("TRN2", target_bir_lowering=False, debug=False)

    # host_perm: the host pre-permutes each shard to [tiles//tpd, P, tpd*C]
    # (group-major, partition-major) so every load/store is a fully
    # contiguous 2D AP — tpd*2KiB per partition per descriptor instead of
    # tpd separate 2KiB runs. The SBUF-side layout is identical.
    assert not (dma_bitcast and not host_perm)
    ddt = FP if dma_bitcast else XDT          # DRAM-side declared dtype
    dcol = (tpd * C) // 2 if dma_bitcast else tpd * C
    if host_perm:
        x_d = nc.dram_tensor("x", [tiles // tpd, P, dcol], ddt, kind="ExternalInput")
    else:
        x_d = nc.dram_tensor("x", [tiles, P, C], XDT, kind="ExternalInput")
    a_d = nc.dram_tensor("amat", [N, P], FP, kind="ExternalInput")
    be_d = nc.dram_tensor("bexp", [P, 1], FP, kind="ExternalInput")
    dg_d = nc.dram_tensor("daug", [P, N + 1], FP, kind="ExternalInput")
    b2_d = nc.dram_tensor("b2", [N, 1], FP, kind="ExternalInput")
    if host_perm:
        o_d = nc.dram_tensor("out", [tiles // tpd, P, dcol], ddt, kind="ExternalOutput")
    else:
        o_d = nc.dram_tensor("out", [tiles, P, C], XDT, kind="ExternalOutput")

    with tile.TileContext(nc) as tc, ExitStack() as ctx:
        const = ctx.enter_context(tc.tile_pool(name="const", bufs=1))
        xp = ctx.enter_context(
            tc.tile_pool(name="xp", bufs=xbufs or (2 * g // tpd))
        )
        yp = ctx.enter_context(tc.tile_pool(name="yp", bufs=mlp_bufs))
        sp = ctx.enter_context(tc.tile_pool(name="sp", bufs=mlp_bufs))
        svp = ctx.enter_context(tc.tile_pool(name="svp", bufs=2 * g))
        sdp = ctx.enter_context(tc.tile_pool(name="sdp", bufs=2))
        # 3 PSUM tags (f1/zs/rb) x bufs must fit 8 banks -> cap at 2
        pp = ctx.enter_context(
            tc.tile_pool(name="pp", bufs=min(mlp_bufs, 2), space="PSUM")
        )

        a_sb = const.tile([N, P], FP)
        nc.sync.dma_start(a_sb[:], a_d[:])
        be_sb = const.tile([P, 1], FP)
        nc.sync.dma_start(be_sb[:], be_d[:])
        dg_sb = const.tile([P, N + 1], FP)
        nc.sync.dma_start(dg_sb[:], dg_d[:])
        b2_sb = const.tile([N, 1], FP)
        nc.sync.dma_start(b2_sb[:], b2_d[:])
        ones_sb = const.tile([1, N], FP)
        nc.vector.memset(ones_sb[:], 1.0)

        loop_cm = tc.For_i(0, repeat, 1) if repeat > 1 else None
        if loop_cm is not None:
            loop_cm.__enter__()

        st_eng = {"scalar": nc.scalar, "sync": nc.sync, "gpsimd": nc.gpsimd}[
            store_engine
        ]
        do_mlp = ablate is None or ablate == "nomul"
        for ch in range(nchunk):
            y_coll = yp.tile([P, g], XDT if lowp_ycoll else FP)
            xts = []
            for i in range(0, g, tpd):
                t = ch * g + i
                xt = xp.tile([P, tpd * C], XDT)
                xt3 = xt[:].rearrange("p (d c) -> p d c", d=tpd)
                if host_perm:
                    dst = xt[:].bitcast(FP) if dma_bitcast else xt[:]
                    nc.sync.dma_start(dst, x_d[t // tpd])
                else:
                    nc.sync.dma_start(
                        xt3, x_d[t : t + tpd].rearrange("d p c -> p d c")
                    )
                if do_mlp:
                    if lowp_ycoll:
                        with nc.allow_low_precision(
                            reason="bf16 row-sums: |err|~2^-9 on y feeds a "
                            "sigmoid-bounded scale; rel-err budget is 2e-2"
                        ):
                            nc.vector.reduce_sum(
                                y_coll[:, i : i + tpd], xt3,
                                axis=mybir.AxisListType.X,
                            )
                    else:
                        nc.vector.reduce_sum(
                            y_coll[:, i : i + tpd], xt3, axis=mybir.AxisListType.X
                        )
                xts.append(xt)

            if do_mlp:
                y_all = sp.tile([N, 2 * g], FP)
                nc.vector.tensor_copy(y_all[:, 0:g], y_coll[0:N, :])
                nc.vector.tensor_copy(y_all[:, g : 2 * g], y_coll[N:P, :])

                f1 = pp.tile([P, 2 * g], FP)
                nc.tensor.matmul(f1[:], a_sb[:], y_all[:])
                e_sb = sp.tile([P, 2 * g], FP)
                nc.scalar.activation(
                    e_sb[:], f1[:], mybir.ActivationFunctionType.Exp,
                    bias=be_sb[:], scale=SCALE,
                )
                zs = pp.tile([N + 1, 2 * g], FP)
                nc.tensor.matmul(zs[:], dg_sb[:], e_sb[:])
                rs = sp.tile([1, 2 * g], FP)
                nc.vector.reciprocal(rs[:], zs[N : N + 1, :])
                rb = pp.tile([N, 2 * g], FP)
                nc.tensor.matmul(rb[:], ones_sb[:], rs[:])
                rb_sb = sp.tile([N, 2 * g], FP)
                nc.vector.tensor_copy(rb_sb[:], rb[:])
                zn = sp.tile([N, 2 * g], FP)
                nc.vector.tensor_mul(zn[:], zs[0:N, :], rb_sb[:])
                y2 = sp.tile([N, 2 * g], FP)
                if SIGMOID_EXP:
                    # b2_sb holds -b2; e2 = exp(-(zn + b2)), y2 = 1/(1+e2)
                    e2 = sp.tile([N, 2 * g], FP)
                    nc.scalar.activation(
                        e2[:], zn[:], mybir.ActivationFunctionType.Exp,
                        bias=b2_sb[:], scale=-1.0,
                    )
                    t2 = sp.tile([N, 2 * g], FP)
                    nc.vector.tensor_scalar_add(t2[:], e2[:], 1.0)
                    nc.vector.reciprocal(y2[:], t2[:])
                else:
                    nc.scalar.activation(
                        y2[:], zn[:], mybir.ActivationFunctionType.Sigmoid,
                        bias=b2_sb[:],
                    )

            # per-chunk mul plan: tiles k < k_act on ACT (per-column Copy with
            # per-partition scale), tiles k >= k_act on DVE (one broadcast
            # tensor_mul per tpd-block sub-range)
            if mul_engine == "slab":
                k_act = 0
            elif mul_engine.startswith("split:"):
                k_act = int(mul_engine.split(":")[1])
            else:
                k_act = g

            svc = svd = None
            if do_mlp and ablate is None:
                sv_eng = getattr(nc, sv_engine)
                if k_act > 0:
                    # svc[(r,j), i] = y2[j, r*g + i]
                    svc = svp.tile([P, g], FP)
                    sv_eng.tensor_copy(svc[0:N, :], y2[:, 0:g])
                    sv_eng.tensor_copy(svc[N:P, :], y2[:, g : 2 * g])
                if k_act < g:
                    # separate scale tile for the DVE slab muls
                    svd = sdp.tile([P, g], XDT if lowp_sv else FP)
                    nc.vector.tensor_copy(svd[0:N, :], y2[:, 0:g])
                    nc.vector.tensor_copy(svd[N:P, :], y2[:, g : 2 * g])

            for i in range(0, g, tpd):
                t = ch * g + i
                xt = xts[i // tpd]
                if ablate == "constmul":
                    for u in range(tpd):
                        col = xt[:, u * C : (u + 1) * C]
                        nc.scalar.activation(
                            col, col,
                            mybir.ActivationFunctionType.Copy, scale=0.5,
                        )
                elif ablate is None:
                    for u in range(min(max(k_act - i, 0), tpd)):
                        col = xt[:, u * C : (u + 1) * C]
                        if mul_engine == "vector":
                            nc.vector.tensor_scalar_mul(
                                col, col, svc[:, i + u : i + u + 1]
                            )
                        else:
                            nc.scalar.activation(
                                col, col,
                                mybir.ActivationFunctionType.Copy,
                                scale=svc[:, i + u : i + u + 1],
                            )
                    u0 = min(max(k_act - i, 0), tpd)
                    if u0 < tpd:
                        xt3m = xt[:].rearrange("p (d c) -> p d c", d=tpd)
                        svb = (
                            svd[:, i + u0 : i + tpd]
                            .unsqueeze(2)
                            .broadcast_to([P, tpd - u0, C])
                        )
                        nc.vector.tensor_mul(
                            xt3m[:, u0:tpd, :], xt3m[:, u0:tpd, :], svb
                        )
                if host_perm:
                    src = xt[:].bitcast(FP) if dma_bitcast else xt[:]
                    if fine_tail and ch == nchunk - 1:
                        # last chunk: stream stores out in 2-tile pieces as
                        # their muls land, shortening the serial kernel tail
                        hc = C // 2 if dma_bitcast else C
                        for s0 in range(0, tpd, 2):
                            st_eng.dma_start(
                                o_d[t // tpd][:, s0 * hc : (s0 + 2) * hc],
                                src[:, s0 * hc : (s0 + 2) * hc],
                            )
                    else:
                        st_eng.dma_start(o_d[t // tpd], src)
                else:
                    st_eng.dma_start(
                        o_d[t : t + tpd].rearrange("d p c -> p d c"),
                        xt[:].rearrange("p (d c) -> p d c", d=tpd),
                    )

        if loop_cm is not None:
            loop_cm.__exit__(None, None, None)

    nc.compile()
    return nc


def _prep_weights(w1, b1, w2, b2, mb):
    w1 = np.asarray(w1, np.float64)
    b1 = np.asarray(b1, np.float64)
    w2 = np.asarray(w2, np.float64)
    b2 = np.asarray(b2, np.float64)
    mb = np.asarray(mb, np.float64)
    a = np.ascontiguousarray(((w1.T @ mb) / C).astype(np.float32))
    be = np.ascontiguousarray(((b1 @ mb) * SCALE).astype(np.float32).reshape(P, 1))
    dg = np.concatenate([(w2 @ mb).T, np.ones((P, 1))], axis=1)
    dg = np.ascontiguousarray(dg.astype(np.float32))
    b2s = -b2 if SIGMOID_EXP else b2
    b2c = np.ascontiguousarray(b2s.astype(np.float32).reshape(N, 1))
    return a, be, dg, b2c


DMA_BITCAST = False  # DMA x/out as fp32-typed APs (same bytes on the wire)


def prep_x_shards(x, bitcast=None):
    """fp32 (b,N,Nwin,p,n,c) -> per-core bf16 shards in the module's layout."""
    if bitcast is None:
        bitcast = DMA_BITCAST
    xs = np.asarray(x, np.float32).reshape(N_CORES, TILES, P, C).astype(XNP)
    if HOST_PERM:
        # group-major, partition-major packing: every on-device DMA becomes
        # one contiguous TPD*1KiB run per partition (see _build_module)
        xs = np.ascontiguousarray(
            xs.reshape(N_CORES, TILES // TPD, TPD, P, C).transpose(0, 1, 3, 2, 4)
        ).reshape(N_CORES, TILES // TPD, P, TPD * C)
    if bitcast:
        xs = xs.view(np.float32)  # same bytes, fp32-typed for the DMA APs
    return xs


def prep_in_maps(x, w1, b1, w2, b2, mb, bitcast=None):
    a, be, dg, b2c = _prep_weights(w1, b1, w2, b2, mb)
    xs = prep_x_shards(x, bitcast=bitcast)
    return [
        {"x": xs[i], "amat": a, "bexp": be, "daug": dg, "b2": b2c}
        for i in range(N_CORES)
    ]


def kernel(x, w1, b1, w2, b2, mb):
    global _CACHED
    x = np.asarray(x, np.float32)
    b, Nn, Nwin, p, n, c = x.shape

    if _CACHED is None:
        _CACHED = _build_module()
    nc = _CACHED

    in_maps = prep_in_maps(x, w1, b1, w2, b2, mb)
    global LAST_RESULTS
    LAST_RESULTS = run_bass_kernel_spmd(
        nc, in_maps, core_ids=list(range(N_CORES)),
        trace=bool(os.environ.get("KERNEL_TRACE")),
    )
    res = LAST_RESULTS.results
    out = np.stack([r["out"] for r in res], axis=0)
    if DMA_BITCAST:
        out = out.view(XNP)
    if HOST_PERM:
        out = np.ascontiguousarray(
            out.reshape(N_CORES, TILES // TPD, P, TPD, C).transpose(0, 1, 3, 2, 4)
        )
    return out.reshape(b, Nn, Nwin, p, n, c).astype(np.float32)


if __name__ == "__main__":
    xt = np.random.randn(2, 16, 16, 4, 64, 512).astype(np.float32)
    w1t = (np.random.randn(32, 64) * 0.1).astype(np.float32)
    b1t = (np.random.randn(32) * 0.1).astype(np.float32)
    w2t = (np.random.randn(64, 32) * 0.1).astype(np.float32)
    b2t = (np.random.randn(64) * 0.1).astype(np.float32)
    mbt = np.random.randn(32, 128).astype(np.float32)
    o = kernel(xt, w1t, b1t, w2t, b2t, mbt)
    print(o.shape, o.dtype)

